# revision 46
# baseline (speedup 1.0000x reference)
"""SAGAN-style self-attention block on 8 trn2 NeuronCores.

Sharding: core = (b, half) with b = core // 2 (batch), half = core % 2
(query-row half of the image). Each core receives x[b] rearranged to
[128, 8192]: partitions 0-63 hold the core's own 64 image rows
(queries + residual), partitions 64-127 hold the other 64 rows (needed
only for the pooled keys/values phi/g). All cores run the identical
program (pure SPMD); the per-core view differs only through the input
arrays.

Per-core dataflow (C=64, N_half=8192 queries, M=4096 pooled keys):
  conv:  fused 1x1-conv matmuls in bf16 (1 cyc/row on PE); weight
         layout [g(32) | phi(8) | theta(8)]. theta/phi stored bf16 and
         replicated per-chunk onto the three PE row strips (ACT/DVE/GP
         copies, overlapped with the conv matmuls). 2x2 maxpool of
         phi/g via bf16 tensor_max (DVE 2x/4x modes); g transposed
         chunkwise on PE into gt (bf16) with a ones column (softmax
         denominator).
  attn:  per 512-query block, groups of 3 key-chunks, software
         pipelined: S^T chunk [128m, 512n] via bf16 K=8 matmuls
         row-tiled to 3 PE strips; exp on ScalarE (PSUM -> SBUF bf16);
         the o-side matmuls of group g-1 are emitted after the S
         matmuls of group g so PE never waits on ACT.
  tail:  merged into the attention loop one block behind: denominator
         copy (GP) -> reciprocal_approx_fast (DVE) -> gamma/denom
         broadcast via K=1 bf16 outer product reusing the S PSUM pool
         -> output conv w_o (bf16) -> residual add -> DMA out.
"""

import numpy as np

_CACHE = {}

C = 64
NHALF = 8192  # queries per core
M = 4096  # pooled key positions
NB = 16  # 512-query blocks
MCH = 32  # 128-wide m-chunks
GRP = 2  # m-chunks per exp group = one DoubleRow pair (2 PSUM banks each)


def _split_multiwaits(nc):
    """This walrus build accepts only one sync-wait per instruction, but
    Tile can attach several (e.g. a matmul fanning in from two DMA lanes,
    or the exit drain collecting one wait per DMA lane).  Hoist all but
    one wait onto standalone EventSemaphore carriers on the same engine,
    placed immediately before the instruction."""
    import concourse.mybir as mybir

    for f in nc.m.functions:
        for bb in f.blocks:
            out = []
            changed = False
            for ins in bb.instructions:
                si = getattr(ins, "sync_info", None)
                conds = list(si.on_wait) if si is not None and si.on_wait else []
                if len(conds) > 1:
                    for c in conds[:-1]:
                        es = mybir.InstNoOp(
                            name=nc.get_next_instruction_name(), ins=[], outs=[]
                        )
                        es.engine = ins.engine
                        es.sync_info = mybir.SyncInfo(on_wait=[c], on_update=[])
                        nc.register_instruction(es, overwrite=True)
                        out.append(es)
                    si.on_wait = [conds[-1]]
                    changed = True
                out.append(ins)
            if changed:
                bb.instructions = out


def _build(gamma: float):
    import concourse.bass as bass
    import concourse.mybir as mybir
    from concourse import tile

    f32 = mybir.dt.float32
    bf16 = mybir.dt.bfloat16
    f8 = mybir.dt.float8e4
    DR = mybir.MatmulPerfMode.DoubleRow
    Exp = mybir.ActivationFunctionType.Exp

    nc = bass.Bass()
    xl_d = nc.dram_tensor("xl", [128, NHALF], bf16, kind="ExternalInput")
    wall_d = nc.dram_tensor("wall", [128, 48], bf16, kind="ExternalInput")
    wot_d = nc.dram_tensor("wot", [32, 64], bf16, kind="ExternalInput")
    id_d = nc.dram_tensor("ident", [32, 32], f32, kind="ExternalInput")
    out_d = nc.dram_tensor("out", [128, NHALF // 2], f32, kind="ExternalOutput")

    with tile.TileContext(nc) as tc:
        with (
            tc.tile_pool(name="consts", bufs=1) as cpool,
            tc.tile_pool(name="xin", bufs=4) as xpool,
            tc.tile_pool(name="big", bufs=1) as bpool,
        ):
            wall_sb = cpool.tile([128, 48], bf16, tag="wall")
            wot_sb = cpool.tile([32, 64], bf16, tag="wot")
            id_sb = cpool.tile([32, 32], f32, tag="ident")
            gam1 = cpool.tile([1, 32], bf16, tag="gam1")

            xl = [
                xpool.tile([128, 2048], bf16, tag="xl", name=f"xl{i}")
                for i in range(4)
            ]

            theta = bpool.tile([72, NHALF], bf16, tag="theta")
            phi = bpool.tile([72, M], bf16, tag="phi")
            g_sb = bpool.tile([32, M], f32, tag="g")
            gt = bpool.tile([128, 64 * MCH], f8, tag="gt")

            nc.sync.dma_start(out=wall_sb[:], in_=wall_d[:])
            nc.sync.dma_start(out=wot_sb[:], in_=wot_d[:])
            nc.sync.dma_start(out=id_sb[:], in_=id_d[:])
            for i in range(4):
                nc.sync.dma_start(out=xl[i][:], in_=xl_d[:, 2048 * i : 2048 * (i + 1)])
            nc.vector.memset(gam1[:], gamma)
            nc.vector.memset(gt[:], 0.0)
            # ones column at slot 32 of every 64-wide chunk (softmax denom)
            nc.vector.memset(
                gt[:].rearrange("q (c w) -> q c w", w=64)[:, :, 32:33], 1.0
            )

            # ---- conv + pool + g-transpose phase -------------------------
            with (
                tc.tile_pool(name="cpsum", bufs=3, space="PSUM") as cps,
                tc.tile_pool(name="thpsum", bufs=2, space="PSUM") as hps,
                tc.tile_pool(name="tpsum", bufs=2, space="PSUM") as tps,
                tc.tile_pool(name="scr", bufs=3) as scr,
            ):
                for t in range(16):
                    xi, xo = t // 4, (t % 4) * 512

                    def pool40(psrc, moff, cidx, on_act):
                        # psrc rows 0-31 = g, 32-39 = phi; covers 4 image
                        # rows x 128 cols.  2x2 maxpool: PSUM -> SBUF
                        # bf16 staging copy (engines can read only one
                        # PSUM operand per instruction; ACT and DVE
                        # alternate), then two strided bf16 tensor_max
                        # steps; partition bases stay 32-aligned.
                        sc = scr.tile([40, 512], bf16, tag="sc")
                        if on_act:
                            nc.scalar.copy(sc[:], psrc[0:40, :])
                        else:
                            nc.vector.tensor_copy(sc[:], psrc[0:40, :])
                        s1 = scr.tile([40, 256], bf16, tag="s1")
                        v1 = sc[:].rearrange("p (a two) -> p a two", two=2)
                        nc.vector.tensor_max(s1[:], v1[:, :, 0], v1[:, :, 1])
                        v2 = s1[:].rearrange(
                            "p (r two c) -> p r two c", two=2, c=64
                        )
                        nc.vector.tensor_max(
                            g_sb[0:32, moff : moff + 128].rearrange(
                                "p (r c) -> p r c", c=64
                            ),
                            v2[0:32, :, 0, :],
                            v2[0:32, :, 1, :],
                        )
                        nc.vector.tensor_max(
                            phi[32:40, moff : moff + 128].rearrange(
                                "p (r c) -> p r c", c=64
                            ),
                            v2[32:40, :, 0, :],
                            v2[32:40, :, 1, :],
                        )
                        # replicate the phi chunk to the other strip
                        # (DMA: free engine bandwidth, latency hidden)
                        nc.sync.dma_start(
                            out=phi[0:8, moff : moff + 128],
                            in_=phi[32:40, moff : moff + 128],
                        )
                        # transpose this g chunk into gt (ones col stays)
                        pt = tps.tile([128, 32], f32, tag="gtp")
                        nc.tensor.transpose(
                            pt[:], g_sb[0:32, moff : moff + 128], id_sb[:]
                        )
                        nc.scalar.copy(gt[:, 64 * cidx : 64 * cidx + 32], pt[:])

                    # own half: g+phi conv, theta conv (separate psum)
                    pa = cps.tile([40, 512], f32, tag="conv")
                    nc.tensor.matmul(
                        pa[:],
                        wall_sb[0:64, 0:40],
                        xl[xi][0:64, xo : xo + 512],
                        start=True,
                        stop=True,
                        tile_position=(0, 0),
                    )
                    pt8 = hps.tile([8, 512], f32, tag="th")
                    nc.tensor.matmul(
                        pt8[:],
                        wall_sb[0:64, 40:48],
                        xl[xi][0:64, xo : xo + 512],
                        start=True,
                        stop=True,
                        tile_position=(0, 0),
                    )
                    # theta: ACT copies PSUM->bf16 row strip 0; DMA
                    # replicates to strips 1/2.
                    nc.scalar.copy(theta[0:8, 512 * t : 512 * t + 512], pt8[0:8, :])
                    nc.sync.dma_start(
                        out=theta[32:40, 512 * t : 512 * t + 512],
                        in_=theta[0:8, 512 * t : 512 * t + 512],
                    )
                    pool40(pa, 128 * t, t, on_act=True)

                    # other half: g+phi only
                    pb = cps.tile([40, 512], f32, tag="conv")
                    nc.tensor.matmul(
                        pb[:],
                        wall_sb[64:128, 0:40],
                        xl[xi][64:128, xo : xo + 512],
                        start=True,
                        stop=True,
                        tile_position=(64, 0),
                    )
                    pool40(pb, 2048 + 128 * t, 16 + t, on_act=False)

            # ---- attention + tail (software pipelined) ------------------
            groups = []
            mc0 = 0
            while mc0 < MCH:
                groups.append((mc0, min(GRP, MCH - mc0)))
                mc0 += GRP
            NG = len(groups)

            with (
                tc.tile_pool(name="spsum", bufs=3, space="PSUM") as sps,
                tc.tile_pool(name="opsum", bufs=2, space="PSUM") as ops,
                tc.tile_pool(name="epool", bufs=4) as ep,
                tc.tile_pool(name="fsb", bufs=2) as fsb,
            ):
                def emit_S(nb, gi, mid=None):
                    # mid() emits the lagged o-matmul BETWEEN the two S
                    # strip matmuls, so the PE's in-flight slots alternate
                    # S/o instead of draining fully between phases.
                    g0, gsz = groups[gi]
                    ps = sps.tile([128, 512 * GRP], f32, tag="ps", name=f"ps{nb}_{gi}")
                    for j in range(gsz):
                        mc = g0 + j
                        nc.tensor.matmul(
                            ps[:, 512 * j : 512 * j + 512],
                            phi[32 * j : 32 * j + 8, 128 * mc : 128 * mc + 128],
                            theta[32 * j : 32 * j + 8, 512 * nb : 512 * nb + 512],
                            start=True,
                            stop=True,
                            tile_position=(32 * j, 0),
                        )
                        if j == 0 and mid is not None:
                            mid()
                    return ps

                # Schraudolph fast-exp constants targeting the fp8e4m3
                # bit pattern: int8(s * 2^3/ln2 + (7*2^3 - 366000/2^20)).
                # Scores span [-4.2, 4.2] -> bits in [7, 105], safely
                # inside [0, 126] (127 would be NaN).  ~4% mean error on
                # 1/3 of the keys; softmax self-normalization shrinks the
                # net effect far below budget.
                EA8 = float(2.0**3 / np.log(2.0))
                EB8 = float(7 * 2**3 - 366000.0 / 2**20)
                i8 = mybir.dt.int8

                def emit_exp(nb, gi, ps):
                    # one group = one DoubleRow pair (chunks 2gi, 2gi+1).
                    # Every third group's exp runs on DVE (fast-exp into
                    # fp8e4m3 bit patterns); the rest on ACT.
                    et = ep.tile([128, 1024], f8, tag="et", name=f"et{nb}_{gi}")
                    if gi % 3 == 2:
                        nc.vector.tensor_scalar(
                            et[:].bitcast(i8),
                            ps[:, 0:1024],
                            EA8,
                            EB8,
                            mybir.AluOpType.mult,
                            mybir.AluOpType.add,
                        )
                    else:
                        nc.scalar.activation(et[:], ps[:, 0:1024], Exp)
                    return et

                def emit_o_pair(p, et, po):
                    # one fp8 DoubleRow matmul accumulates TWO key-chunks
                    # (pair p = chunks 2p, 2p+1) into po.
                    nc.tensor.matmul(
                        po[:],
                        gt[:, 128 * p : 128 * p + 128].rearrange(
                            "q (k m) -> q k m", k=2
                        ),
                        et[:, 0:1024].rearrange("q (k n) -> q k n", k=2),
                        start=(p == 0),
                        stop=(p == MCH // 2 - 1),
                        perf_mode=DR,
                        skip_group_check=True,
                    )

                # tail state carried one block behind
                pend = {}

                def emit_tail_head(nb, po):
                    # block end: denominator + magic-constant reciprocal
                    # seed only (keep the block-boundary DVE chain short
                    # so the next block's fast-exp ops aren't delayed).
                    den = fsb.tile([1, 512], f32, tag="den", name=f"den{nb}")
                    nc.vector.tensor_copy(den[:], po[32:33, :])
                    i32 = mybir.dt.int32
                    r0i = fsb.tile([1, 512], i32, tag="r0i", name=f"r0i{nb}")
                    nc.vector.tensor_scalar(
                        r0i[:],
                        den[:].bitcast(i32),
                        -1,
                        0x7EF311C3,
                        mybir.AluOpType.mult,
                        mybir.AluOpType.add,
                    )
                    pend["nb"] = nb
                    pend["po"] = po
                    pend["den"] = den
                    pend["r0i"] = r0i

                def emit_tail_om(state):
                    # next block, early: stage numerator out of the (still
                    # live, double-buffered) po bank.
                    nb = state["nb"]
                    om = fsb.tile([32, 512], f32, tag="om", name=f"om{nb}")
                    nc.vector.tensor_copy(om[:], state["po"][0:32, :])
                    state["om"] = om

                def emit_tail_newton(state):
                    # one Newton step; final output lands in bf16 for the
                    # broadcast matmul.
                    nb = state["nb"]
                    r0 = state["r0i"][:].bitcast(f32)
                    u = fsb.tile([1, 512], f32, tag="u", name=f"u{nb}")
                    nc.vector.scalar_tensor_tensor(
                        u[:], state["den"][:], -1.0, r0,
                        mybir.AluOpType.mult, mybir.AluOpType.mult,
                    )
                    rb = fsb.tile([1, 512], bf16, tag="rb", name=f"rb{nb}")
                    nc.vector.scalar_tensor_tensor(
                        rb[:], u[:], 2.0, r0,
                        mybir.AluOpType.add, mybir.AluOpType.mult,
                    )
                    state["rb"] = rb

                def emit_tail_rep(state):
                    # gamma/denom broadcast to 32 partitions via K=1 bf16
                    # outer product; PSUM comes from the S pool (bank
                    # reuse, no extra PSUM).
                    nb = state["nb"]
                    rep = sps.tile([32, 512], f32, tag="ps", name=f"rep{nb}")
                    nc.tensor.matmul(
                        rep[:],
                        gam1[:],
                        state["rb"][:],
                        start=True,
                        stop=True,
                        tile_position=(0, 0),
                    )
                    omn = fsb.tile([32, 512], bf16, tag="omn", name=f"omn{nb}")
                    nc.vector.tensor_mul(omn[:], state["om"][:], rep[:])
                    state["omn"] = omn

                def emit_tail_oc(state):
                    nb = state["nb"]
                    oc = sps.tile([64, 512], f32, tag="ps", name=f"oc{nb}")
                    nc.tensor.matmul(
                        oc[:], wot_sb[:], state["omn"][:], start=True, stop=True
                    )
                    stage = fsb.tile([64, 512], f32, tag="stage", name=f"stage{nb}")
                    nc.vector.tensor_add(
                        stage[:],
                        oc[:],
                        xl[nb // 4][0:64, (nb % 4) * 512 : (nb % 4) * 512 + 512],
                    )
                    pp = 0 if nb < 8 else 64
                    off = 512 * nb if nb < 8 else 512 * (nb - 8)
                    nc.sync.dma_start(
                        out=out_d[pp : pp + 64, off : off + 512], in_=stage[:]
                    )

                for nb in range(NB):
                    po = ops.tile([64, 512], f32, tag="po", name=f"po{nb}")
                    ets = {}
                    for gi in range(NG):
                        mid = None
                        if gi >= 2:
                            mid = lambda g=gi: emit_o_pair(
                                g - 2, ets.pop(g - 2), po
                            )
                        ps = emit_S(nb, gi, mid)
                        ets[gi] = emit_exp(nb, gi, ps)
                        if gi == 1 and pend.get("r0i") is not None:
                            emit_tail_om(pend)
                        if gi == 3 and pend.get("om") is not None:
                            emit_tail_newton(pend)
                        if gi == 6 and pend.get("rb") is not None:
                            emit_tail_rep(pend)
                        if gi == 9 and pend.get("omn") is not None:
                            emit_tail_oc(pend)
                            pend.clear()
                    emit_o_pair(NG - 2, ets.pop(NG - 2), po)
                    emit_o_pair(NG - 1, ets.pop(NG - 1), po)
                    emit_tail_head(nb, po)

                # flush the final block's tail
                emit_tail_om(pend)
                emit_tail_newton(pend)
                emit_tail_rep(pend)
                emit_tail_oc(pend)

    _split_multiwaits(nc)
    return nc


def _get_program(gamma: float):
    key = float(gamma)
    if key not in _CACHE:
        _CACHE[key] = _build(key)
    return _CACHE[key]


def _make_in_maps(x, w_theta, w_phi, w_g, w_o):
    import ml_dtypes

    bf = ml_dtypes.bfloat16
    x = np.asarray(x, np.float32)
    w_theta = np.asarray(w_theta, np.float32)
    w_phi = np.asarray(w_phi, np.float32)
    w_g = np.asarray(w_g, np.float32)
    w_o = np.asarray(w_o, np.float32)
    B, C_, H, W = x.shape
    # weight column layout: [g(32) | phi(8) | theta(8)]
    w_all = np.concatenate([w_g.T, w_phi.T, w_theta.T], axis=1)  # [64, 48]
    wall2 = np.ascontiguousarray(np.concatenate([w_all, w_all], axis=0)).astype(bf)
    wot = np.ascontiguousarray(w_o.T).astype(bf)
    ident = np.eye(32, dtype=np.float32)
    in_maps = []
    for core in range(8):
        b, half = core // 2, core % 2
        xb = x[b].reshape(C_, H, W)
        xo = xb[:, 64 * half : 64 * half + 64, :].reshape(C_, NHALF)
        xr = xb[:, 64 * (1 - half) : 64 * (1 - half) + 64, :].reshape(C_, NHALF)
        xlc = np.ascontiguousarray(np.concatenate([xo, xr], axis=0)).astype(bf)
        in_maps.append({"xl": xlc, "wall": wall2, "wot": wot, "ident": ident})
    return in_maps


def _assemble(results, B, C_, H, W):
    out = np.zeros((B, C_, H, W), np.float32)
    for core in range(8):
        b, half = core // 2, core % 2
        o = np.asarray(results[core]["out"])  # [128, 4096]
        oh = np.concatenate([o[0:64, :], o[64:128, :]], axis=1)  # [64, 8192]
        out[b, :, 64 * half : 64 * half + 64, :] = oh.reshape(C_, 64, W)
    return out


def kernel(x, w_theta, w_phi, w_g, w_o, gamma, _trace=False):
    from concourse.bass_utils import run_bass_kernel_spmd

    x = np.asarray(x, np.float32)
    gamma_f = float(np.asarray(gamma))
    nc = _get_program(gamma_f)
    in_maps = _make_in_maps(x, w_theta, w_phi, w_g, w_o)
    res = run_bass_kernel_spmd(nc, in_maps, list(range(8)), trace=_trace)
    out = _assemble(res.results, *x.shape)
    if _trace:
        kernel._last_result = res
    return out


# revision 47
# speedup vs baseline: 1.2362x; 1.2362x over previous
"""SAGAN-style self-attention block on 8 trn2 NeuronCores.

Sharding: core = (b, half) with b = core // 2 (batch), half = core % 2
(query-row half of the image). Each core receives x[b] rearranged to
[128, 8192]: partitions 0-63 hold the core's own 64 image rows
(queries + residual), partitions 64-127 hold the other 64 rows (needed
only for the pooled keys/values phi/g). All cores run the identical
program (pure SPMD); the per-core view differs only through the input
arrays.

Per-core dataflow (C=64, N_half=8192 queries, M=4096 pooled keys):
  conv:  fused 1x1-conv matmuls in bf16 (1 cyc/row on PE); weight
         layout [g(32) | phi(8) | theta(8)]. theta/phi stored bf16 and
         replicated per-chunk onto the three PE row strips (ACT/DVE/GP
         copies, overlapped with the conv matmuls). 2x2 maxpool of
         phi/g via bf16 tensor_max (DVE 2x/4x modes); g transposed
         chunkwise on PE into gt (bf16) with a ones column (softmax
         denominator).
  attn:  per 512-query block, groups of 3 key-chunks, software
         pipelined: S^T chunk [128m, 512n] via bf16 K=8 matmuls
         row-tiled to 3 PE strips; exp on ScalarE (PSUM -> SBUF bf16);
         the o-side matmuls of group g-1 are emitted after the S
         matmuls of group g so PE never waits on ACT.
  tail:  merged into the attention loop one block behind: denominator
         copy (GP) -> reciprocal_approx_fast (DVE) -> gamma/denom
         broadcast via K=1 bf16 outer product reusing the S PSUM pool
         -> output conv w_o (bf16) -> residual add -> DMA out.
"""

import numpy as np

_CACHE = {}

C = 64
NHALF = 8192  # queries per core
M = 4096  # pooled key positions
NB = 16  # 512-query blocks
MCH = 32  # 128-wide m-chunks
GRP = 2  # m-chunks per exp group = one DoubleRow pair (2 PSUM banks each)


def _split_multiwaits(nc):
    """This walrus build accepts only one sync-wait per instruction, but
    Tile can attach several (e.g. a matmul fanning in from two DMA lanes,
    or the exit drain collecting one wait per DMA lane).  Hoist all but
    one wait onto standalone EventSemaphore carriers on the same engine,
    placed immediately before the instruction."""
    import concourse.mybir as mybir

    for f in nc.m.functions:
        for bb in f.blocks:
            out = []
            changed = False
            for ins in bb.instructions:
                si = getattr(ins, "sync_info", None)
                conds = list(si.on_wait) if si is not None and si.on_wait else []
                if len(conds) > 1:
                    for c in conds[:-1]:
                        es = mybir.InstNoOp(
                            name=nc.get_next_instruction_name(), ins=[], outs=[]
                        )
                        es.engine = ins.engine
                        es.sync_info = mybir.SyncInfo(on_wait=[c], on_update=[])
                        nc.register_instruction(es, overwrite=True)
                        out.append(es)
                    si.on_wait = [conds[-1]]
                    changed = True
                out.append(ins)
            if changed:
                bb.instructions = out


def _build(gamma: float):
    import concourse.bass as bass
    import concourse.mybir as mybir
    from concourse import tile

    f32 = mybir.dt.float32
    bf16 = mybir.dt.bfloat16
    f8 = mybir.dt.float8e4
    DR = mybir.MatmulPerfMode.DoubleRow
    Exp = mybir.ActivationFunctionType.Exp

    nc = bass.Bass()
    xl_d = nc.dram_tensor("xl", [128, NHALF], bf16, kind="ExternalInput")
    wall_d = nc.dram_tensor("wall", [128, 48], bf16, kind="ExternalInput")
    wot_d = nc.dram_tensor("wot", [32, 64], bf16, kind="ExternalInput")
    id_d = nc.dram_tensor("ident", [32, 32], f32, kind="ExternalInput")
    out_d = nc.dram_tensor("out", [128, NHALF // 2], f32, kind="ExternalOutput")

    with tile.TileContext(nc) as tc:
        with (
            tc.tile_pool(name="consts", bufs=1) as cpool,
            tc.tile_pool(name="xin", bufs=4) as xpool,
            tc.tile_pool(name="big", bufs=1) as bpool,
        ):
            wall_sb = cpool.tile([128, 48], bf16, tag="wall")
            wot_sb = cpool.tile([32, 64], bf16, tag="wot")
            id_sb = cpool.tile([32, 32], f32, tag="ident")
            gam1 = cpool.tile([1, 32], bf16, tag="gam1")

            xl = [
                xpool.tile([128, 2048], bf16, tag="xl", name=f"xl{i}")
                for i in range(4)
            ]

            theta = bpool.tile([72, NHALF], bf16, tag="theta")
            phi = bpool.tile([72, M], bf16, tag="phi")
            g_sb = bpool.tile([32, M], f32, tag="g")
            gt = bpool.tile([128, 64 * MCH], f8, tag="gt")

            nc.sync.dma_start(out=wall_sb[:], in_=wall_d[:])
            nc.sync.dma_start(out=wot_sb[:], in_=wot_d[:])
            nc.sync.dma_start(out=id_sb[:], in_=id_d[:])
            for i in range(4):
                nc.sync.dma_start(out=xl[i][:], in_=xl_d[:, 2048 * i : 2048 * (i + 1)])
            nc.vector.memset(gam1[:], gamma)
            nc.vector.memset(gt[:], 0.0)
            # ones column at slot 32 of every 64-wide chunk (softmax denom)
            nc.vector.memset(
                gt[:].rearrange("q (c w) -> q c w", w=64)[:, :, 32:33], 1.0
            )

            # ---- conv + pool + g-transpose phase -------------------------
            with (
                tc.tile_pool(name="cpsum", bufs=3, space="PSUM") as cps,
                tc.tile_pool(name="thpsum", bufs=2, space="PSUM") as hps,
                tc.tile_pool(name="tpsum", bufs=2, space="PSUM") as tps,
                tc.tile_pool(name="scr", bufs=3) as scr,
            ):
                for t in range(16):
                    xi, xo = t // 4, (t % 4) * 512

                    def pool40(psrc, moff, cidx, on_act):
                        # psrc rows 0-31 = g, 32-39 = phi; covers 4 image
                        # rows x 128 cols.  2x2 maxpool: PSUM -> SBUF
                        # bf16 staging copy (engines can read only one
                        # PSUM operand per instruction; ACT and DVE
                        # alternate), then two strided bf16 tensor_max
                        # steps; partition bases stay 32-aligned.
                        sc = scr.tile([40, 512], bf16, tag="sc")
                        if on_act:
                            nc.scalar.copy(sc[:], psrc[0:40, :])
                        else:
                            nc.vector.tensor_copy(sc[:], psrc[0:40, :])
                        s1 = scr.tile([40, 256], bf16, tag="s1")
                        v1 = sc[:].rearrange("p (a two) -> p a two", two=2)
                        nc.vector.tensor_max(s1[:], v1[:, :, 0], v1[:, :, 1])
                        v2 = s1[:].rearrange(
                            "p (r two c) -> p r two c", two=2, c=64
                        )
                        nc.vector.tensor_max(
                            g_sb[0:32, moff : moff + 128].rearrange(
                                "p (r c) -> p r c", c=64
                            ),
                            v2[0:32, :, 0, :],
                            v2[0:32, :, 1, :],
                        )
                        nc.vector.tensor_max(
                            phi[32:40, moff : moff + 128].rearrange(
                                "p (r c) -> p r c", c=64
                            ),
                            v2[32:40, :, 0, :],
                            v2[32:40, :, 1, :],
                        )
                        # replicate the phi chunk to the other strip
                        # (DMA: free engine bandwidth, latency hidden)
                        nc.sync.dma_start(
                            out=phi[0:8, moff : moff + 128],
                            in_=phi[32:40, moff : moff + 128],
                        )
                        # transpose this g chunk into gt (ones col stays)
                        pt = tps.tile([128, 32], f32, tag="gtp")
                        nc.tensor.transpose(
                            pt[:], g_sb[0:32, moff : moff + 128], id_sb[:]
                        )
                        nc.scalar.copy(gt[:, 64 * cidx : 64 * cidx + 32], pt[:])

                    # own half: g+phi conv, theta conv (separate psum)
                    pa = cps.tile([40, 512], f32, tag="conv")
                    nc.tensor.matmul(
                        pa[:],
                        wall_sb[0:64, 0:40],
                        xl[xi][0:64, xo : xo + 512],
                        start=True,
                        stop=True,
                        tile_position=(0, 0),
                    )
                    pt8 = hps.tile([8, 512], f32, tag="th")
                    nc.tensor.matmul(
                        pt8[:],
                        wall_sb[0:64, 40:48],
                        xl[xi][0:64, xo : xo + 512],
                        start=True,
                        stop=True,
                        tile_position=(0, 0),
                    )
                    # theta: ACT copies PSUM->bf16 row strip 0; DMA
                    # replicates to strips 1/2.
                    nc.scalar.copy(theta[0:8, 512 * t : 512 * t + 512], pt8[0:8, :])
                    nc.sync.dma_start(
                        out=theta[32:40, 512 * t : 512 * t + 512],
                        in_=theta[0:8, 512 * t : 512 * t + 512],
                    )
                    pool40(pa, 128 * t, t, on_act=True)

                    # other half: g+phi only
                    pb = cps.tile([40, 512], f32, tag="conv")
                    nc.tensor.matmul(
                        pb[:],
                        wall_sb[64:128, 0:40],
                        xl[xi][64:128, xo : xo + 512],
                        start=True,
                        stop=True,
                        tile_position=(64, 0),
                    )
                    pool40(pb, 2048 + 128 * t, 16 + t, on_act=False)

            # ---- attention + tail (software pipelined) ------------------
            groups = []
            mc0 = 0
            while mc0 < MCH:
                groups.append((mc0, min(GRP, MCH - mc0)))
                mc0 += GRP
            NG = len(groups)

            with (
                tc.tile_pool(name="spsum", bufs=3, space="PSUM") as sps,
                tc.tile_pool(name="opsum", bufs=2, space="PSUM") as ops,
                tc.tile_pool(name="epool", bufs=4) as ep,
                tc.tile_pool(name="fsb", bufs=2) as fsb,
            ):
                def emit_S(nb, gi):
                    g0, gsz = groups[gi]
                    ps = sps.tile([128, 512 * GRP], f32, tag="ps", name=f"ps{nb}_{gi}")
                    for j in range(gsz):
                        mc = g0 + j
                        nc.tensor.matmul(
                            ps[:, 512 * j : 512 * j + 512],
                            phi[32 * j : 32 * j + 8, 128 * mc : 128 * mc + 128],
                            theta[32 * j : 32 * j + 8, 512 * nb : 512 * nb + 512],
                            start=True,
                            stop=True,
                            tile_position=(32 * j, 0),
                        )
                    return ps

                # Schraudolph fast-exp constants targeting the fp8e4m3
                # bit pattern: int8(s * 2^3/ln2 + (7*2^3 - 366000/2^20)).
                # Scores span [-4.2, 4.2] -> bits in [7, 105], safely
                # inside [0, 126] (127 would be NaN).  ~4% mean error on
                # 1/3 of the keys; softmax self-normalization shrinks the
                # net effect far below budget.
                EA8 = float(2.0**3 / np.log(2.0))
                EB8 = float(7 * 2**3 - 366000.0 / 2**20)
                i8 = mybir.dt.int8

                def emit_exp(nb, gi, ps):
                    # one group = one DoubleRow pair (chunks 2gi, 2gi+1).
                    # Every third group's exp runs on DVE (fast-exp into
                    # fp8e4m3 bit patterns); the rest on ACT.
                    et = ep.tile([128, 1024], f8, tag="et", name=f"et{nb}_{gi}")
                    if gi % 3 == 2:
                        nc.vector.tensor_scalar(
                            et[:].bitcast(i8),
                            ps[:, 0:1024],
                            EA8,
                            EB8,
                            mybir.AluOpType.mult,
                            mybir.AluOpType.add,
                        )
                    else:
                        nc.scalar.activation(et[:], ps[:, 0:1024], Exp)
                    return et

                def emit_o_pair(p, et, po):
                    # one fp8 DoubleRow matmul accumulates TWO key-chunks
                    # (pair p = chunks 2p, 2p+1) into po.
                    nc.tensor.matmul(
                        po[:],
                        gt[:, 128 * p : 128 * p + 128].rearrange(
                            "q (k m) -> q k m", k=2
                        ),
                        et[:, 0:1024].rearrange("q (k n) -> q k n", k=2),
                        start=(p == 0),
                        stop=(p == MCH // 2 - 1),
                        perf_mode=DR,
                        skip_group_check=True,
                    )

                # tail state carried one block behind
                pend = {}

                def emit_tail_head(nb, po):
                    # block end: denominator + magic-constant reciprocal
                    # seed only (keep the block-boundary DVE chain short
                    # so the next block's fast-exp ops aren't delayed).
                    den = fsb.tile([1, 512], f32, tag="den", name=f"den{nb}")
                    nc.vector.tensor_copy(den[:], po[32:33, :])
                    i32 = mybir.dt.int32
                    r0i = fsb.tile([1, 512], i32, tag="r0i", name=f"r0i{nb}")
                    nc.vector.tensor_scalar(
                        r0i[:],
                        den[:].bitcast(i32),
                        -1,
                        0x7EF311C3,
                        mybir.AluOpType.mult,
                        mybir.AluOpType.add,
                    )
                    pend["nb"] = nb
                    pend["po"] = po
                    pend["den"] = den
                    pend["r0i"] = r0i

                def emit_tail_om(state):
                    # next block, early: stage numerator out of the (still
                    # live, double-buffered) po bank.
                    nb = state["nb"]
                    om = fsb.tile([32, 512], f32, tag="om", name=f"om{nb}")
                    nc.vector.tensor_copy(om[:], state["po"][0:32, :])
                    state["om"] = om

                def emit_tail_newton(state):
                    # one Newton step; final output lands in bf16 for the
                    # broadcast matmul.
                    nb = state["nb"]
                    r0 = state["r0i"][:].bitcast(f32)
                    u = fsb.tile([1, 512], f32, tag="u", name=f"u{nb}")
                    nc.vector.scalar_tensor_tensor(
                        u[:], state["den"][:], -1.0, r0,
                        mybir.AluOpType.mult, mybir.AluOpType.mult,
                    )
                    rb = fsb.tile([1, 512], bf16, tag="rb", name=f"rb{nb}")
                    nc.vector.scalar_tensor_tensor(
                        rb[:], u[:], 2.0, r0,
                        mybir.AluOpType.add, mybir.AluOpType.mult,
                    )
                    state["rb"] = rb

                def emit_tail_rep(state):
                    # gamma/denom broadcast to 32 partitions via K=1 bf16
                    # outer product; PSUM comes from the S pool (bank
                    # reuse, no extra PSUM).
                    nb = state["nb"]
                    rep = sps.tile([32, 512], f32, tag="ps", name=f"rep{nb}")
                    nc.tensor.matmul(
                        rep[:],
                        gam1[:],
                        state["rb"][:],
                        start=True,
                        stop=True,
                        tile_position=(0, 0),
                    )
                    omn = fsb.tile([32, 512], bf16, tag="omn", name=f"omn{nb}")
                    nc.vector.tensor_mul(omn[:], state["om"][:], rep[:])
                    state["omn"] = omn

                def emit_tail_oc(state):
                    nb = state["nb"]
                    oc = sps.tile([64, 512], f32, tag="ps", name=f"oc{nb}")
                    nc.tensor.matmul(
                        oc[:], wot_sb[:], state["omn"][:], start=True, stop=True
                    )
                    stage = fsb.tile([64, 512], f32, tag="stage", name=f"stage{nb}")
                    nc.vector.tensor_add(
                        stage[:],
                        oc[:],
                        xl[nb // 4][0:64, (nb % 4) * 512 : (nb % 4) * 512 + 512],
                    )
                    pp = 0 if nb < 8 else 64
                    off = 512 * nb if nb < 8 else 512 * (nb - 8)
                    nc.sync.dma_start(
                        out=out_d[pp : pp + 64, off : off + 512], in_=stage[:]
                    )

                for nb in range(NB):
                    po = ops.tile([64, 512], f32, tag="po", name=f"po{nb}")
                    ets = {}
                    for gi in range(NG):
                        ps = emit_S(nb, gi)
                        ets[gi] = emit_exp(nb, gi, ps)
                        if gi == 1 and pend.get("r0i") is not None:
                            emit_tail_om(pend)
                        if gi == 3 and pend.get("om") is not None:
                            emit_tail_newton(pend)
                        if gi == 6 and pend.get("rb") is not None:
                            emit_tail_rep(pend)
                        if gi == 9 and pend.get("omn") is not None:
                            emit_tail_oc(pend)
                            pend.clear()
                        # o-side lags two groups behind its exp
                        if gi >= 2:
                            emit_o_pair(gi - 2, ets.pop(gi - 2), po)
                    emit_o_pair(NG - 2, ets.pop(NG - 2), po)
                    emit_o_pair(NG - 1, ets.pop(NG - 1), po)
                    emit_tail_head(nb, po)

                # flush the final block's tail
                emit_tail_om(pend)
                emit_tail_newton(pend)
                emit_tail_rep(pend)
                emit_tail_oc(pend)

    _split_multiwaits(nc)
    return nc


def _get_program(gamma: float):
    key = float(gamma)
    if key not in _CACHE:
        _CACHE[key] = _build(key)
    return _CACHE[key]


def _make_in_maps(x, w_theta, w_phi, w_g, w_o):
    import ml_dtypes

    bf = ml_dtypes.bfloat16
    x = np.asarray(x, np.float32)
    w_theta = np.asarray(w_theta, np.float32)
    w_phi = np.asarray(w_phi, np.float32)
    w_g = np.asarray(w_g, np.float32)
    w_o = np.asarray(w_o, np.float32)
    B, C_, H, W = x.shape
    # weight column layout: [g(32) | phi(8) | theta(8)]
    w_all = np.concatenate([w_g.T, w_phi.T, w_theta.T], axis=1)  # [64, 48]
    wall2 = np.ascontiguousarray(np.concatenate([w_all, w_all], axis=0)).astype(bf)
    wot = np.ascontiguousarray(w_o.T).astype(bf)
    ident = np.eye(32, dtype=np.float32)
    in_maps = []
    for core in range(8):
        b, half = core // 2, core % 2
        xb = x[b].reshape(C_, H, W)
        xo = xb[:, 64 * half : 64 * half + 64, :].reshape(C_, NHALF)
        xr = xb[:, 64 * (1 - half) : 64 * (1 - half) + 64, :].reshape(C_, NHALF)
        xlc = np.ascontiguousarray(np.concatenate([xo, xr], axis=0)).astype(bf)
        in_maps.append({"xl": xlc, "wall": wall2, "wot": wot, "ident": ident})
    return in_maps


def _assemble(results, B, C_, H, W):
    out = np.zeros((B, C_, H, W), np.float32)
    for core in range(8):
        b, half = core // 2, core % 2
        o = np.asarray(results[core]["out"])  # [128, 4096]
        oh = np.concatenate([o[0:64, :], o[64:128, :]], axis=1)  # [64, 8192]
        out[b, :, 64 * half : 64 * half + 64, :] = oh.reshape(C_, 64, W)
    return out


def kernel(x, w_theta, w_phi, w_g, w_o, gamma, _trace=False):
    from concourse.bass_utils import run_bass_kernel_spmd

    x = np.asarray(x, np.float32)
    gamma_f = float(np.asarray(gamma))
    nc = _get_program(gamma_f)
    in_maps = _make_in_maps(x, w_theta, w_phi, w_g, w_o)
    res = run_bass_kernel_spmd(nc, in_maps, list(range(8)), trace=_trace)
    out = _assemble(res.results, *x.shape)
    if _trace:
        kernel._last_result = res
    return out
